# revision 1
# baseline (speedup 1.0000x reference)
"""Causal depthwise Conv1d (K=4) + SiLU on 8 Trainium2 NeuronCores.

Problem: x (4, 8192, 2048) f32, w (2048, 1, 4) f32 ->
         y = silu(causal_depthwise_conv1d(x, w)) (4, 8192, 2048) f32.

Sharding: pure data parallel over (batch, seq-half): core c handles batch c//2,
seq rows [ (c%2)*4096, (c%2)*4096+4096 ). The K-1=3 halo is shipped with each
shard (4099 seq positions), so cores are fully independent — no collectives.

Per-core layout: the host transposes each shard to (D, S) = (2048, 4099) so
DMAs are contiguous along the free (seq) dim. On chip: channels on the 128
partitions, seq on the free dim. The conv runs on the TensorEngine as 4
accumulating matmuls per tile with 128x128 diagonal weight matrices (one per
tap) in float32r (full-rate fp32 PE mode, ~2^-12 rounding), accumulated in
PSUM in fp32; the ScalarEngine applies SiLU on the PSUM -> SBUF move.

The diagonal weight matrices are built fully on-device: a 128x128 identity
from memset + gpsimd affine_select (iota col-row == 0), scaled by DVE
tensor_scalar_mul with per-partition weight columns — only the 32 KB weight
table crosses HBM instead of 4 MB. x loads use the SP HWDGE ring; y stores
alternate between the gpsimd SWDGE path and the ACT HWDGE ring so descriptor
generation for loads and stores proceeds in parallel. A dummy Silu at kernel
start preloads the ACT table set (~2.7 us) under the pipeline fill.

TimelineSim: 190.1 us/core = 186.6 us DMA transfer (67 MB @ 360 GB/s, 98.2%
occupancy, zero steady-state gaps) + 2.0 us DGE-pipeline head + 1.5 us drain
tail. Both head and tail are fixed framework/hardware latencies; DMA bytes
are minimal (x read once incl. one K-1 halo per row, y written once).

Execution uses a locally-cached jax.jit(shard_map) built once per process
(bass2jax.run_bass_via_pjrt rebuilds and retraces it per call).
"""

import time

import numpy as np

import concourse.bass as bass  # noqa: F401  (registers bass_rust bindings)
import concourse.mybir as mybir
import concourse.tile as tile
from concourse import bacc

B, S, D, K = 4, 8192, 2048, 4
NCORES = 8
SH = S // 2            # seq rows per core
SPAD = SH + K - 1      # shard seq width incl. halo
P = 128                # SBUF partitions
DB = D // P            # channel blocks per core
TS = 512               # seq tile (= one PSUM bank of fp32)
NTILE = SH // TS

VERBOSE = False        # set by test.py for phase timings

_cached = None         # cached jitted runner
_cached_nc = None      # cached compiled Bass program


def _build_nc():
    global _cached_nc
    if _cached_nc is not None:
        return _cached_nc
    f32 = mybir.dt.float32
    f32r = mybir.dt.float32r

    nc = bacc.Bacc(
        trn_type="TRN2",
        target_bir_lowering=False,
        debug=False,
        num_devices=NCORES,
    )
    # x is declared float32r (same bits as f32); the PE rounds on read.
    xt_d = nc.dram_tensor("xt", [D, SPAD], f32r, kind="ExternalInput").ap()
    wc_d = nc.dram_tensor("wc", [P, DB * K], f32, kind="ExternalInput").ap()
    yt_d = nc.dram_tensor("yt", [D, SH], f32, kind="ExternalOutput").ap()

    with tile.TileContext(nc) as tc:
        with (
            tc.tile_pool(name="wp", bufs=1) as wpool,
            tc.tile_pool(name="xp", bufs=3) as xpool,
            tc.tile_pool(name="yp", bufs=3) as ypool,
            tc.tile_pool(name="ps", bufs=8, space="PSUM") as pspool,
        ):
            wc_t = wpool.tile([P, DB * K], f32)
            nc.scalar.dma_start(wc_t[:], wc_d)

            # Preload the Silu ACT table set under the pipeline fill.
            scratch = wpool.tile([P, 1], f32)
            nc.vector.memset(scratch[:], 0.0)
            nc.scalar.activation(scratch[:], scratch[:],
                                 mybir.ActivationFunctionType.Silu)

            # On-device 128x128 identity: ones, then zero where col != row.
            eye_t = wpool.tile([P, P], f32)
            nc.vector.memset(eye_t[:], 1.0)
            nc.gpsimd.affine_select(eye_t[:], eye_t[:], [[1, P]],
                                    mybir.AluOpType.is_equal, 0.0,
                                    channel_multiplier=-1)

            # Build the 64 diagonal 128x128 tap matrices: diag(w[j*128:+128, 0, k]).
            wsb = wpool.tile([P, DB * K * P], f32r)
            for jk in range(DB * K):
                nc.vector.tensor_scalar_mul(
                    wsb[:, jk * P:(jk + 1) * P], eye_t[:], wc_t[:, jk:jk + 1])

            # x loads are fine-grained (512-wide) so compute starts as soon as
            # each slice lands; each j-row shares one (128, 4099) buffer with
            # the K-1 halo shipped exactly once (disjoint sub-loads; matmul
            # windows span slice boundaries — Tile range-deps handle it).
            # y stores ship 1 MB halves per row. The first six x loads
            # alternate SP-HWDGE / SWDGE so the two DGE pipelines' startup
            # latencies overlap; the last row's stores are split per-tile so
            # the final bytes ship as soon as each ACT ends.
            YST = 1024           # y store width (1 MB)
            n = 0
            nx = 0
            for j in range(DB):
                xg = xpool.tile([P, SPAD], f32r)
                nc.sync.dma_start(
                    xg[:, 0: TS + K - 1], xt_d[j * P:(j + 1) * P, 0: TS + K - 1])
                for q in range(1, NTILE):
                    c0 = (K - 1) + q * TS
                    x_eng = nc.gpsimd if (nx < 6 and nx % 2 == 1) else nc.sync
                    x_eng.dma_start(
                        xg[:, c0: c0 + TS],
                        xt_d[j * P:(j + 1) * P, c0: c0 + TS])
                    nx += 1
                y_t = ypool.tile([P, SH], f32)
                for ui in range(NTILE):
                    ps = pspool.tile([P, TS], f32)
                    for k in range(K):
                        c0 = (j * K + k) * P
                        nc.tensor.matmul(
                            ps[:],
                            wsb[:, c0:c0 + P],
                            xg[:, ui * TS + k: ui * TS + k + TS],
                            start=(k == 0),
                            stop=(k == K - 1),
                        )
                    nc.scalar.activation(
                        y_t[:, ui * TS:(ui + 1) * TS], ps[:],
                        mybir.ActivationFunctionType.Silu)
                if j == DB - 1:
                    for q in range(NTILE):
                        y_eng = nc.gpsimd if q % 2 == 0 else nc.scalar
                        y_eng.dma_start(
                            yt_d[j * P:(j + 1) * P, q * TS:(q + 1) * TS],
                            y_t[:, q * TS:(q + 1) * TS])
                else:
                    for q in range(SH // YST):
                        y_eng = nc.gpsimd if n % 2 == 0 else nc.scalar
                        y_eng.dma_start(
                            yt_d[j * P:(j + 1) * P, q * YST:(q + 1) * YST],
                            y_t[:, q * YST:(q + 1) * YST])
                        n += 1
    nc.compile()
    _cached_nc = nc
    return nc


def _get_runner():
    """Build (once) a cached jax.jit(shard_map) executing the Bass program on
    8 cores. Mirrors bass2jax.run_bass_via_pjrt's multi-core path, but the
    jitted callable survives across kernel() calls (the library rebuilds and
    retraces it per invocation)."""
    global _cached
    if _cached is not None:
        return _cached

    import jax
    from jax.sharding import Mesh, PartitionSpec
    from jax.experimental.shard_map import shard_map
    from concourse import bass2jax

    bass2jax.install_neuronx_cc_hook()

    nc = _build_nc()

    in_names = ["xt", "wc"]
    out_names = ["yt"]
    out_avals = (jax.core.ShapedArray((D, SH), np.float32),)
    all_names = in_names + out_names + ["partition_id"]
    n_params = len(in_names)

    def _body(*args):
        operands = list(args)
        operands.append(bass2jax.partition_id_tensor())
        outs = bass2jax._bass_exec_p.bind(
            *operands,
            out_avals=out_avals,
            in_names=tuple(all_names),
            out_names=tuple(out_names),
            lowering_input_output_aliases=(),
            sim_require_finite=True,
            sim_require_nnan=True,
            nc=nc,
        )
        return tuple(outs)

    devices = jax.devices()[:NCORES]
    mesh = Mesh(np.asarray(devices), ("core",))
    n_args = n_params + len(out_names)
    sharded = jax.jit(
        shard_map(
            _body,
            mesh=mesh,
            in_specs=(PartitionSpec("core"),) * n_args,
            out_specs=(PartitionSpec("core"),) * len(out_names),
            check_rep=False,
        ),
        donate_argnums=(n_params,),
        keep_unused=True,
    )
    _cached = sharded
    return sharded


def kernel(x: np.ndarray, w: np.ndarray) -> np.ndarray:
    t0 = time.time()
    sharded = _get_runner()
    t_build = time.time() - t0

    x = np.asarray(x, dtype=np.float32)
    w = np.asarray(w, dtype=np.float32)

    t0 = time.time()
    # wc[p, j*K + k] = w[j*128 + p, 0, k]
    wc1 = np.ascontiguousarray(
        w[:, 0, :].reshape(DB, P, K).transpose(1, 0, 2).reshape(P, DB * K))
    wc = np.broadcast_to(wc1, (NCORES, P, DB * K)).reshape(NCORES * P, DB * K)

    # Concatenated per-core transposed shards: (8*2048, 4099)
    xt = np.zeros((NCORES * D, SPAD), dtype=np.float32)
    for c in range(NCORES):
        b, h = divmod(c, 2)
        s0 = h * SH
        lo = s0 - (K - 1)
        dst = xt[c * D:(c + 1) * D]
        if lo < 0:
            dst[:, K - 1 - s0:] = x[b, 0: s0 + SH, :].T
        else:
            dst[:, :] = x[b, lo: s0 + SH, :].T
    zeros = np.zeros((NCORES * D, SH), dtype=np.float32)
    t_prep = time.time() - t0

    t0 = time.time()
    (out,) = sharded(xt, wc, zeros)
    t_run = time.time() - t0

    # Fetch the 8 output shards in parallel (the d2h tunnel is the wall-clock
    # bottleneck; concurrent per-device fetches overlap it) and un-transpose.
    t0 = time.time()
    import concurrent.futures as cf

    y = np.empty((B, S, D), dtype=np.float32)

    def _fetch(sh):
        c = sh.index[0].start // D
        b, h = divmod(c, 2)
        s0 = h * SH
        y[b, s0: s0 + SH, :] = np.asarray(sh.data).T

    with cf.ThreadPoolExecutor(NCORES) as ex:
        list(ex.map(_fetch, out.addressable_shards))
    t_post = time.time() - t0

    if VERBOSE:
        print(f"[kernel] build {t_build:.2f}s prep {t_prep:.2f}s "
              f"run {t_run:.2f}s post {t_post:.2f}s", flush=True)
    return y



# revision 2
# speedup vs baseline: 1.8409x; 1.8409x over previous
"""Causal depthwise Conv1d (K=4) + SiLU on 8 Trainium2 NeuronCores.

Problem: x (4, 8192, 2048) f32, w (2048, 1, 4) f32 ->
         y = silu(causal_depthwise_conv1d(x, w)) (4, 8192, 2048) f32.

Sharding: pure data parallel over (batch, seq-half): core c handles batch c//2,
seq rows [ (c%2)*4096, (c%2)*4096+4096 ). The K-1=3 halo is shipped with each
shard, so cores are fully independent — no collectives.

v2 over the f32 baseline (190.1 us): all device I/O is fp16 (error budget:
abs-max/absmax metric tolerates 2e-2; fp16 end-to-end lands ~1e-3), halving
HBM traffic to ~33.6 MB/core -> ~93.5 us DMA at the 360 GB/s cost-model
roofline. That makes the PE the next bottleneck (4 diagonal matmuls/tile =
109 us), so the conv uses 2-way sequence packing: each partition holds one
(channel, seq-parity) pair — block b covers 64 channels x 2 parities; a
128x128 stationary then applies TWO taps per matmul via intra-block bands,
so each 512-col psum tile needs only ceil((K-1)/2)+1 = 3 accumulating
matmuls (PE 82 us < DMA). SiLU runs on ACT over paired psum banks
(64 x [128,1024], ~66 us), writing fp16 directly.

The three banded stationaries per block are built on-device with 6 DVE
tensor_scalar_muls from (a) a [128, 64] staircase mask M[p, q] = (q == p//2)
built once via gpsimd affine_select, and (b) a tiny host-shipped weight
table wc[p, 5b+i] holding the per-(parity, tap-pair) weight columns with
validity masking pre-applied. Only ~80 KB of weight/table data crosses HBM.

Roofline: 33.6 MB/core @ 360 GB/s = 93.5 us DMA transfer + ~2 us DGE head
+ ~1.5 us drain tail. PE 58%, ACT ~75%, DVE ~40% busy — all under DMA.

Execution uses a locally-cached jax.jit(shard_map) built once per process.
"""

import time

import numpy as np

import concourse.bass as bass  # noqa: F401  (registers bass_rust bindings)
import concourse.mybir as mybir
import concourse.tile as tile
from concourse import bacc

B, S, D, K = 4, 8192, 2048, 4
NCORES = 8
SH = S // 2            # out seq rows per core
P = 128                # SBUF partitions
R = 2                  # seq positions packed per partition group
CPB = P // R           # channels per block (64)
NBLK = D // CPB        # channel blocks per core (32)
TC = SH // R           # packed seq cols (2048)
HC = 2                 # halo cols (ceil((K-1)/R))
XC = TC + HC           # x cols incl. halo (2050)
TS = 512               # psum tile cols (one bank of fp32)
NT = TC // TS          # psum tiles per block (4)
NM = (K - 1 + R - 1) // R + 1   # matmuls per tile (3)
WPB = 2 * NM - 1       # weight-table cols per block (5; m2 j1 is all-zero)
WCOLS = NBLK * WPB + 1  # + shared zero column

VERBOSE = False        # set by test.py for phase timings

_cached = None         # cached jitted runner
_cached_nc = None      # cached compiled Bass program


def _build_nc():
    global _cached_nc
    if _cached_nc is not None:
        return _cached_nc
    f16 = mybir.dt.float16
    f32 = mybir.dt.float32

    nc = bacc.Bacc(
        trn_type="TRN2",
        target_bir_lowering=False,
        debug=False,
        num_devices=NCORES,
    )
    xt_d = nc.dram_tensor("xt", [NBLK * P, XC], f16, kind="ExternalInput").ap()
    wc_d = nc.dram_tensor("wc", [P, WCOLS], f32, kind="ExternalInput").ap()
    yt_d = nc.dram_tensor("yt", [NBLK * P, TC], f16, kind="ExternalOutput").ap()

    with tile.TileContext(nc) as tc:
        with (
            tc.tile_pool(name="wp", bufs=1) as wpool,
            tc.tile_pool(name="xp", bufs=4) as xpool,
            tc.tile_pool(name="yp", bufs=4) as ypool,
            tc.tile_pool(name="s0p", bufs=4) as s0pool,
            tc.tile_pool(name="s1p", bufs=4) as s1pool,
            tc.tile_pool(name="s2p", bufs=4) as s2pool,
            tc.tile_pool(name="ps", bufs=4, space="PSUM") as pspool,
        ):
            wc_t = wpool.tile([P, WCOLS], f32)
            nc.scalar.dma_start(wc_t[:], wc_d)

            # Preload the Silu ACT table set under the pipeline fill.
            scratch = wpool.tile([P, 1], f32)
            nc.vector.memset(scratch[:], 0.0)
            nc.scalar.activation(scratch[:], scratch[:],
                                 mybir.ActivationFunctionType.Silu)

            # Staircase mask M[p, q] = 1 iff q == p // R  (i.e. the moving
            # row (channel, parity) p belongs to staircase channel q).
            m64 = wpool.tile([P, CPB], f16)
            nc.vector.memset(m64[:], 1.0)
            nc.gpsimd.affine_select(m64[:], m64[:], [[R, CPB]],
                                    mybir.AluOpType.is_ge, 0.0,
                                    channel_multiplier=-1)
            nc.gpsimd.affine_select(m64[:], m64[:], [[R, CPB]],
                                    mybir.AluOpType.is_le, 0.0,
                                    base=-(R - 1), channel_multiplier=-1)

            nx = 0
            for b in range(NBLK):
                # x rows for this block: 64 channels x 2 parities, with the
                # K-1 halo shipped as HC packed leading cols. Two ~1 MB-class
                # chunk loads; the first few alternate SP-HWDGE / SWDGE so
                # both DGE pipelines' startup latencies overlap.
                xg = xpool.tile([P, XC], f16)
                for (c0, c1) in ((0, XC // 2 + 1), (XC // 2 + 1, XC)):
                    x_eng = nc.gpsimd if (nx < 6 and nx % 2 == 1) else nc.sync
                    x_eng.dma_start(xg[:, c0:c1],
                                    xt_d[b * P:(b + 1) * P, c0:c1])
                    nx += 1

                # Banded stationaries S_m[p_in, (q, j)] = w_{R*m + j - p_in%R}
                # [ch(b, p_in//R)] iff q == p_in//R and the tap index is in
                # [0, K); validity masking is pre-applied in the host table.
                s0 = s0pool.tile([P, CPB, R], f16)
                s1 = s1pool.tile([P, CPB, R], f16)
                s2 = s2pool.tile([P, CPB, R], f16)
                c0w = b * WPB
                nc.vector.tensor_scalar_mul(s0[:, :, 0], m64[:],
                                            wc_t[:, c0w + 0:c0w + 1])
                nc.vector.tensor_scalar_mul(s0[:, :, 1], m64[:],
                                            wc_t[:, c0w + 1:c0w + 2])
                nc.vector.tensor_scalar_mul(s1[:, :, 0], m64[:],
                                            wc_t[:, c0w + 2:c0w + 3])
                nc.vector.tensor_scalar_mul(s1[:, :, 1], m64[:],
                                            wc_t[:, c0w + 3:c0w + 4])
                nc.vector.tensor_scalar_mul(s2[:, :, 0], m64[:],
                                            wc_t[:, c0w + 4:c0w + 5])
                nc.vector.tensor_scalar_mul(s2[:, :, 1], m64[:],
                                            wc_t[:, WCOLS - 1:WCOLS])
                smats = (s0, s1, s2)

                y_t = ypool.tile([P, TC], f16)
                for u in range(NT // 2):
                    # Two 512-col accumulation groups in one 2-bank psum
                    # tile; one SiLU over the pair halves ACT's fixed costs.
                    ps = pspool.tile([P, 2 * TS], mybir.dt.float32)
                    for h in range(2):
                        t0 = (u * 2 + h) * TS
                        for m in range(NM):
                            nc.tensor.matmul(
                                ps[:, h * TS:(h + 1) * TS],
                                smats[m][:],
                                xg[:, HC - m + t0: HC - m + t0 + TS],
                                start=(m == 0),
                                stop=(m == NM - 1),
                            )
                    nc.scalar.activation(
                        y_t[:, u * 2 * TS:(u + 1) * 2 * TS], ps[:],
                        mybir.ActivationFunctionType.Silu)

                # y stores alternate SWDGE / ACT-HWDGE; last block splits
                # per-tile so the final bytes ship as soon as each ACT ends.
                if b == NBLK - 1:
                    for q in range(NT):
                        y_eng = nc.gpsimd if q % 2 == 0 else nc.scalar
                        y_eng.dma_start(
                            yt_d[b * P:(b + 1) * P, q * TS:(q + 1) * TS],
                            y_t[:, q * TS:(q + 1) * TS])
                else:
                    for q in range(2):
                        y_eng = nc.gpsimd if (b * 2 + q) % 2 == 0 else nc.scalar
                        y_eng.dma_start(
                            yt_d[b * P:(b + 1) * P,
                                 q * 2 * TS:(q + 1) * 2 * TS],
                            y_t[:, q * 2 * TS:(q + 1) * 2 * TS])
    nc.compile()
    _cached_nc = nc
    return nc


def _get_runner():
    """Build (once) a cached jax.jit(shard_map) executing the Bass program on
    8 cores. Mirrors bass2jax.run_bass_via_pjrt's multi-core path, but the
    jitted callable survives across kernel() calls."""
    global _cached
    if _cached is not None:
        return _cached

    import jax
    from jax.sharding import Mesh, PartitionSpec
    from jax.experimental.shard_map import shard_map
    from concourse import bass2jax

    bass2jax.install_neuronx_cc_hook()

    nc = _build_nc()

    in_names = ["xt", "wc"]
    out_names = ["yt"]
    out_avals = (jax.core.ShapedArray((NBLK * P, TC), np.float16),)
    all_names = in_names + out_names + ["partition_id"]
    n_params = len(in_names)

    def _body(*args):
        operands = list(args)
        operands.append(bass2jax.partition_id_tensor())
        outs = bass2jax._bass_exec_p.bind(
            *operands,
            out_avals=out_avals,
            in_names=tuple(all_names),
            out_names=tuple(out_names),
            lowering_input_output_aliases=(),
            sim_require_finite=True,
            sim_require_nnan=True,
            nc=nc,
        )
        return tuple(outs)

    devices = jax.devices()[:NCORES]
    mesh = Mesh(np.asarray(devices), ("core",))
    n_args = n_params + len(out_names)
    sharded = jax.jit(
        shard_map(
            _body,
            mesh=mesh,
            in_specs=(PartitionSpec("core"),) * n_args,
            out_specs=(PartitionSpec("core"),) * len(out_names),
            check_rep=False,
        ),
        donate_argnums=(n_params,),
        keep_unused=True,
    )
    _cached = sharded
    return sharded


def _make_wtab(w: np.ndarray) -> np.ndarray:
    """Weight table wc[p, b*WPB + i]: the per-partition scalar column for
    build op i of block b. Row p = 2*cl + rin (channel cl within block,
    parity rin); value = w[64b + cl, tap 2m + j - rin] or 0 if the tap
    index falls outside [0, K). i = 2m + j for (m, j) pairs except (2, 1),
    which is the shared all-zero last column."""
    wv = w[:, 0, :].astype(np.float32)          # (D, K)
    Wb = wv.reshape(NBLK, CPB, K)               # [b, cl, k]
    wtab = np.zeros((P, WCOLS), np.float32)
    # (even-row tap, odd-row tap) for i = 0..4
    taps = ((0, None), (1, 0), (2, 1), (3, 2), (None, 3))
    cols = WPB * np.arange(NBLK)
    for i, (ke, ko) in enumerate(taps):
        if ke is not None:
            wtab[0::2, cols + i] = Wb[:, :, ke].T
        if ko is not None:
            wtab[1::2, cols + i] = Wb[:, :, ko].T
    return wtab


def kernel(x: np.ndarray, w: np.ndarray) -> np.ndarray:
    t0 = time.time()
    sharded = _get_runner()
    t_build = time.time() - t0

    x = np.asarray(x, dtype=np.float32)
    w = np.asarray(w, dtype=np.float32)

    t0 = time.time()
    wc1 = _make_wtab(w)
    wc = np.broadcast_to(wc1, (NCORES, P, WCOLS)).reshape(NCORES * P, WCOLS)

    # Per-core packed shards: row b*128 + cl*2 + rin, col = packed seq
    # (HC halo cols + TC), fp16.
    xt = np.empty((NCORES * NBLK * P, XC), dtype=np.float16)
    for c in range(NCORES):
        bb, h = divmod(c, 2)
        s0 = h * SH
        shard = np.zeros((R * XC, D), np.float16)
        lo = s0 - R * HC
        if lo < 0:
            shard[R * HC:] = x[bb, 0:s0 + SH]
        else:
            shard[:] = x[bb, lo:s0 + SH]
        xt[c * NBLK * P:(c + 1) * NBLK * P] = (
            shard.reshape(XC, R, NBLK, CPB)
            .transpose(2, 3, 1, 0).reshape(NBLK * P, XC))
    zeros = np.zeros((NCORES * NBLK * P, TC), dtype=np.float16)
    t_prep = time.time() - t0

    t0 = time.time()
    (out,) = sharded(xt, wc, zeros)
    t_run = time.time() - t0

    # Fetch the 8 output shards in parallel and un-pack.
    t0 = time.time()
    import concurrent.futures as cf

    y = np.empty((B, S, D), dtype=np.float32)

    def _fetch(sh):
        c = sh.index[0].start // (NBLK * P)
        bb, h = divmod(c, 2)
        s0 = h * SH
        ybuf = np.asarray(sh.data)              # (NBLK*P, TC) fp16
        y[bb, s0:s0 + SH, :] = (
            ybuf.reshape(NBLK, CPB, R, TC)
            .transpose(3, 2, 0, 1).reshape(SH, D).astype(np.float32))

    with cf.ThreadPoolExecutor(NCORES) as ex:
        list(ex.map(_fetch, out.addressable_shards))
    t_post = time.time() - t0

    if VERBOSE:
        print(f"[kernel] build {t_build:.2f}s prep {t_prep:.2f}s "
              f"run {t_run:.2f}s post {t_post:.2f}s", flush=True)
    return y
